# revision 1
# baseline (speedup 1.0000x reference)
"""Trainium2 Bass kernel for a 2-layer GCN (GCNConv+ReLU+BN x2, mean-pool).

Strategy (8 NeuronCores, SPMD):
- Dest-node sharding: each core owns NB=ceil(N/8/128) blocks of 128 nodes.
- Message passing out[c] = sum_e w_e * h[src_e] (w_e = dinv[r]*dinv[c],
  self-loops included as edges) is computed per dest block as a chain of
  one-hot matmuls on the TensorEngine: psum += E_t^T @ msg_t, where msg_t is
  128 source rows fetched with dma_gather (fp16 rows padded to 512B) and
  E_t[e, d] = (d_e == d) * w_e built on the VectorEngine via iota/is_equal.
- The weight multiply commutes with aggregation, so x@W never happens
  up front: per block, agg is transposed on the PE and multiplied by W
  (fp16, f32 accum), bias added via a ones-row matmul, then ReLU (ScalarE)
  and the folded BatchNorm affine (VectorE).
- Layer-1 results are written per-core and AllGathered (on-chip collective)
  into a shared buffer that layer 2 gathers from.
- Mean-pool: per block a one-hot P matmul reduces 128 nodes into <=128
  graph partials; the host sums overlapping block partials and divides by
  graph counts (the cross-core unshard step).
"""
import os
import numpy as np
from contextlib import ExitStack

import concourse.bacc as bacc
import concourse.bass as bass
import concourse.mybir as mybir
import concourse.tile as tile
from concourse.library_config import mlp
from concourse.bass_utils import run_bass_kernel_spmd

dt = mybir.dt
NCORES = 8
PB = 128          # nodes per dest block
EW = 256          # fp16 elements per padded row (512 bytes)
EPS = 1e-5
G_OUT = 2048      # number of graphs in the output


# ---------------------------------------------------------------- host prep
def preprocess(x, edge_index, batch, force_T=None):
    N, D = x.shape
    G = G_OUT
    NB = -(-N // (NCORES * PB))          # blocks per core
    S = NB * PB
    TOT = NCORES * S
    L = TOT // 2
    assert L <= 32768 and TOT - L <= 32768

    r = np.asarray(edge_index[0], dtype=np.int64)
    c = np.asarray(edge_index[1], dtype=np.int64)
    loops = np.arange(N, dtype=np.int64)
    r = np.concatenate([r, loops])
    c = np.concatenate([c, loops])
    deg = np.bincount(c, minlength=N).astype(np.float64)
    dinv = np.where(deg > 0, 1.0 / np.sqrt(deg), 0.0)
    w = (dinv[r] * dinv[c]).astype(np.float32)

    half = (r >= L).astype(np.int64)
    src = np.where(half == 0, r, r - L).astype(np.int64)
    blk = c // PB
    d = (c % PB).astype(np.int64)

    ngroups = NCORES * NB * 2
    key = blk * 2 + half
    # sort by (group, src): ascending source rows within each group give the
    # HBM gather far better page locality
    order = np.lexsort((src, key))
    src, d, w, key = src[order], d[order], w[order], key[order]
    counts = np.bincount(key, minlength=ngroups)
    T = max(1, int(-(-counts.max() // PB)))
    if force_T is not None:
        assert force_T >= T
        T = force_T
    gsz = T * PB

    src_p = np.zeros((ngroups, gsz), dtype=np.int16)
    d_p = np.zeros((ngroups, gsz), dtype=np.float32)
    w_p = np.zeros((ngroups, gsz), dtype=np.float32)
    starts = np.concatenate([[0], np.cumsum(counts)])
    for g in range(ngroups):
        n = counts[g]
        src_p[g, :n] = src[starts[g]:starts[g] + n]
        d_p[g, :n] = d[starts[g]:starts[g] + n]
        w_p[g, :n] = w[starts[g]:starts[g] + n]

    # per-core device arrays
    G2 = NB * 2
    idx_arr = np.zeros((NCORES, 128, G2 * 8 * T), dtype=np.int16)
    d_arr = np.zeros((NCORES, 128, G2 * T), dtype=np.float32)
    w_arr = np.zeros((NCORES, 128, G2 * T), dtype=np.float32)
    for k in range(NCORES):
        for gl in range(G2):
            g = k * G2 + gl
            # dma_gather index layout: index j lives at [j % 16, j // 16]
            wrapped = src_p[g].reshape(8 * T, 16).T            # [16, 8T]
            idx_arr[k, :, gl * 8 * T:(gl + 1) * 8 * T] = np.tile(wrapped, (8, 1))
            d_arr[k, :, gl * T:(gl + 1) * T] = d_p[g].reshape(T, PB).T
            w_arr[k, :, gl * T:(gl + 1) * T] = w_p[g].reshape(T, PB).T

    # pooling: per (core, block) graph base + local graph ids
    batch_pad = np.full(TOT, -1, dtype=np.int64)
    batch_pad[:N] = batch
    blocks = batch_pad.reshape(NCORES * NB, PB)
    valid = blocks >= 0
    base = np.where(valid.any(axis=1),
                    np.where(valid, blocks, np.iinfo(np.int64).max).min(axis=1),
                    0)
    bloc = np.where(valid, blocks - base[:, None], -1).astype(np.float32)
    bloc_arr = bloc.reshape(NCORES, NB, PB).transpose(0, 2, 1).copy()  # [k,128,NB]

    cnts = np.bincount(batch, minlength=G).astype(np.float32)
    return dict(N=N, D=D, G=G, NB=NB, S=S, TOT=TOT, L=L, T=T,
                idx_arr=idx_arr, d_arr=d_arr, w_arr=w_arr,
                bloc_arr=bloc_arr, base=base, cnts=cnts)


def fold_bn(g, beta, rm, rv):
    gp = (g / np.sqrt(rv + EPS)).astype(np.float32)
    bp = (beta - rm * gp).astype(np.float32)
    return gp, bp


# ---------------------------------------------------------------- bass build
def build_nc(NB, T, D, TOT, S, L, reps=1, single_core=False):
    f16, f32, i16 = dt.float16, dt.float32, dt.int16
    G2 = NB * 2
    nc = bacc.Bacc("TRN2", target_bir_lowering=False, debug=False,
                   num_devices=1 if single_core else NCORES)

    xbuf = nc.dram_tensor("xbuf", [TOT, EW], f16, kind="ExternalInput")
    idxt = nc.dram_tensor("idxt", [128, G2 * 8 * T], i16, kind="ExternalInput")
    dcol = nc.dram_tensor("dcol", [128, G2 * T], f32, kind="ExternalInput")
    wcol = nc.dram_tensor("wcol", [128, G2 * T], f32, kind="ExternalInput")
    bcol = nc.dram_tensor("bcol", [128, NB], f32, kind="ExternalInput")
    iot = nc.dram_tensor("iot", [128, 128], f16, kind="ExternalInput")
    idn = nc.dram_tensor("idn", [128, 128], f32, kind="ExternalInput")
    onesr = nc.dram_tensor("onesr", [1, 128], f16, kind="ExternalInput")
    whi = nc.dram_tensor("whi", [2, 128, D], f16, kind="ExternalInput")
    wlo = nc.dram_tensor("wlo", [2, D - 128, D], f16, kind="ExternalInput")
    brow = nc.dram_tensor("brow", [2, 1, D], f16, kind="ExternalInput")
    gam = nc.dram_tensor("gam", [2, 128, D], f32, kind="ExternalInput")
    bet = nc.dram_tensor("bet", [2, 128, D], f32, kind="ExternalInput")
    outp = nc.dram_tensor("outp", [NB * PB, D], f32, kind="ExternalOutput")
    h1sl = nc.dram_tensor("h1sl", [S, EW], f16, kind="Internal")
    h1f = nc.dram_tensor("h1f", [TOT, EW], f16, kind="Internal",
                         addr_space="Shared")

    DLO = D - 128
    with tile.TileContext(nc) as tc, ExitStack() as ctx:
        cp = ctx.enter_context(tc.tile_pool(name="consts", bufs=1))
        gp_ = ctx.enter_context(tc.tile_pool(name="gath", bufs=8))
        ep = ctx.enter_context(tc.tile_pool(name="onehot", bufs=4))
        sp = ctx.enter_context(tc.tile_pool(name="stage", bufs=3))
        pagg = ctx.enter_context(tc.tile_pool(name="pagg", bufs=2, space="PSUM"))
        ptr = ctx.enter_context(tc.tile_pool(name="ptr", bufs=2, space="PSUM"))
        ph = ctx.enter_context(tc.tile_pool(name="ph", bufs=2, space="PSUM"))
        ppool = ctx.enter_context(tc.tile_pool(name="ppool", bufs=2, space="PSUM"))

        def load_const(name, dram, shape, dtype):
            t = cp.tile(shape, dtype, name=name)
            nc.sync.dma_start(t[:], dram)
            return t

        idxS = load_const("idxS", idxt[:, :], [128, G2 * 8 * T], i16)
        dS = load_const("dS", dcol[:, :], [128, G2 * T], f32)
        wS = load_const("wS", wcol[:, :], [128, G2 * T], f32)
        bS = load_const("bS", bcol[:, :], [128, NB], f32)
        iotS = load_const("iotS", iot[:, :], [128, 128], f16)
        idnS = load_const("idnS", idn[:, :], [128, 128], f32)
        onesS = load_const("onesS", onesr[:, :], [1, 128], f16)
        whiS = [load_const(f"whiS{l}", whi[l, :, :], [128, D], f16) for l in range(2)]
        wloS = [load_const(f"wloS{l}", wlo[l, :, :], [DLO, D], f16) for l in range(2)]
        browS = [load_const(f"browS{l}", brow[l, :, :], [1, D], f16) for l in range(2)]
        gamS = [load_const(f"gamS{l}", gam[l, :, :], [128, D], f32) for l in range(2)]
        betS = [load_const(f"betS{l}", bet[l, :, :], [128, D], f32) for l in range(2)]

        nc.gpsimd.load_library(mlp)

        for rep in range(reps):
            _do_body(nc, tc, dict(locals(), single_core=single_core))

    nc.compile()
    return nc


def _do_body(nc, tc, env):
    """One full forward pass; env carries the tiles/pools from build_nc."""
    (NB, T, D, TOT, S, L, rep) = (env[k] for k in
                                  ("NB", "T", "D", "TOT", "S", "L", "rep"))
    (xbuf, h1f, h1sl, outp) = (env[k] for k in ("xbuf", "h1f", "h1sl", "outp"))
    (gp_, ep, sp, pagg, ptr, ph, ppool) = (env[k] for k in
        ("gp_", "ep", "sp", "pagg", "ptr", "ph", "ppool"))
    (idxS, dS, wS, bS, iotS, idnS, onesS) = (env[k] for k in
        ("idxS", "dS", "wS", "bS", "iotS", "idnS", "onesS"))
    (whiS, wloS, browS, gamS, betS) = (env[k] for k in
        ("whiS", "wloS", "browS", "gamS", "betS"))
    f16, f32 = dt.float16, dt.float32
    DLO = D - 128
    R = rep
    if True:
        for layer in range(2):
            src = xbuf if layer == 0 else h1f
            for b in range(NB):
                agg = pagg.tile([128, D], f32, name=f"agg_{R}_{layer}_{b}", tag="agg")
                ablate = os.environ.get("ABLATE", "")
                TSUB = int(os.environ.get("GCN_TSUB", "8"))
                for h in range(2):
                    gl = b * 2 + h
                    nsub = -(-T // TSUB)
                    for s_ in range(nsub):
                        t0s = s_ * TSUB
                        tcn = min(T, t0s + TSUB) - t0s
                        gt = gp_.tile([128, tcn, EW], f16,
                                      name=f"gt_{R}_{layer}_{gl}_{s_}", tag="gt")
                        in_ap = src[0:TOT, :] if h == 0 else src[L:TOT, :]
                        if "nogather" not in ablate:
                            nc.gpsimd.dma_gather(
                                gt[:], in_ap,
                                idxS[:, gl * 8 * T + t0s * 8:
                                     gl * 8 * T + (t0s + tcn) * 8],
                                tcn * PB, tcn * PB, EW, single_packet=True)
                        else:
                            nc.vector.memset(gt[:, 0, 0:64], 0)
                        for t in range(t0s, t0s + tcn):
                            cc = gl * T + t
                            if "noE" not in ablate:
                                E = ep.tile([128, 128], f16,
                                            name=f"E_{R}_{layer}_{cc}", tag="E")
                                nc.vector.tensor_scalar(
                                    E[:], iotS[:], dS[:, cc:cc + 1],
                                    wS[:, cc:cc + 1],
                                    op0=mybir.AluOpType.is_equal,
                                    op1=mybir.AluOpType.mult)
                                lhsT = E[:]
                            else:
                                lhsT = iotS[:]
                            if "nomm" not in ablate:
                                nc.tensor.matmul(
                                    agg[:], lhsT, gt[:, t - t0s, 0:D],
                                    start=(h == 0 and t == 0),
                                    stop=(h == 1 and t == T - 1))
                if "nomm" in ablate:
                    nc.tensor.matmul(agg[:], onesS[:], browS[layer][:],
                                     start=True, stop=True)
                # epilogue: transpose agg, @W, +b, relu, BN affine
                aggS = sp.tile([128, D], f32, name=f"aggS_{R}_{layer}_{b}", tag="aggS")
                nc.vector.tensor_copy(aggS[:], agg[:])
                psT = ptr.tile([128, 256], f32, name=f"psT_{R}_{layer}_{b}", tag="psT")
                nc.tensor.transpose(psT[:, 0:128], aggS[:, 0:128], idnS[:])
                nc.tensor.transpose(psT[0:DLO, 128:256], aggS[:, 128:D], idnS[:])
                t1 = sp.tile([128, 128], f16, name=f"t1_{R}_{layer}_{b}", tag="t1")
                nc.vector.tensor_copy(t1[:], psT[:, 0:128])
                t2 = sp.tile([DLO, 128], f16, name=f"t2_{R}_{layer}_{b}", tag="t2")
                nc.vector.tensor_copy(t2[:], psT[0:DLO, 128:256])
                zps = ph.tile([128, D], f32, name=f"zps_{R}_{layer}_{b}", tag="zps")
                nc.tensor.matmul(zps[:], t1[:], whiS[layer][:],
                                 start=True, stop=False)
                nc.tensor.matmul(zps[:], t2[:], wloS[layer][:],
                                 start=False, stop=False)
                nc.tensor.matmul(zps[:], onesS[:], browS[layer][:],
                                 start=False, stop=True)
                rl = sp.tile([128, D], f32, name=f"rl_{R}_{layer}_{b}", tag="rl")
                nc.scalar.activation(rl[:], zps[:],
                                     mybir.ActivationFunctionType.Relu)
                m1 = sp.tile([128, D], f32, name=f"m1_{R}_{layer}_{b}", tag="m1")
                nc.vector.tensor_mul(m1[:], rl[:], gamS[layer][:])
                hS = sp.tile([128, D], f16, name=f"hS_{R}_{layer}_{b}", tag="hS")
                nc.vector.tensor_add(hS[:], m1[:], betS[layer][:])
                if layer == 0:
                    nc.sync.dma_start(h1sl[b * PB:(b + 1) * PB, 0:D], hS[:])
                else:
                    P = ep.tile([128, 128], f16, name=f"P_{R}_{b}", tag="E")
                    nc.vector.tensor_scalar(
                        P[:], iotS[:], bS[:, b:b + 1], None,
                        op0=mybir.AluOpType.is_equal)
                    pps = ppool.tile([128, D], f32, name=f"pps_{R}_{b}", tag="pps")
                    nc.tensor.matmul(pps[:], P[:], hS[:], start=True, stop=True)
                    po = sp.tile([128, D], f32, name=f"po_{R}_{b}", tag="po")
                    nc.vector.tensor_copy(po[:], pps[:])
                    nc.sync.dma_start(outp[b * PB:(b + 1) * PB, :], po[:])
            if layer == 0:
                if env.get("single_core"):
                    # timing variant: stand in for the AllGather with a
                    # plain DMA of this core's slice into the full buffer
                    nc.gpsimd.dma_start(h1f[0:S, :], h1sl[:, :])
                else:
                    nc.gpsimd.collective_compute(
                        "AllGather", mybir.AluOpType.bypass,
                        replica_groups=[list(range(NCORES))],
                        ins=[h1sl[:, :].opt()], outs=[h1f[:, :].opt()])


# ---------------------------------------------------------------- entry
_NC_CACHE = {}


def kernel(x, edge_index, batch, W1, b1, W2, b2,
           g1, beta1, rm1, rv1, g2, beta2, rm2, rv2):
    nc, in_maps, pp = prepare(x, edge_index, batch, W1, b1, W2, b2,
                              g1, beta1, rm1, rv1, g2, beta2, rm2, rv2)
    res = run_bass_kernel_spmd(nc, in_maps, core_ids=list(range(NCORES)))
    return combine(pp, [res.results[k]["outp"] for k in range(NCORES)])


def prepare(x, edge_index, batch, W1, b1, W2, b2,
            g1, beta1, rm1, rv1, g2, beta2, rm2, rv2):
    """Build (nc, in_maps, pp) without running — used by the benchmark."""
    x = np.asarray(x, dtype=np.float32)
    pp = preprocess(x, np.asarray(edge_index), np.asarray(batch))
    D = pp["D"]
    key = (pp["NB"], pp["T"], D, pp["TOT"], pp["S"], pp["L"])
    if key not in _NC_CACHE:
        _NC_CACHE[key] = build_nc(*key)
    nc = _NC_CACHE[key]

    xbuf = np.zeros((pp["TOT"], EW), dtype=np.float16)
    xbuf[:pp["N"], :D] = x.astype(np.float16)
    iot = np.broadcast_to(np.arange(128, dtype=np.float16), (128, 128)).copy()
    idn = np.eye(128, dtype=np.float32)
    onesr = np.ones((1, 128), dtype=np.float16)
    g1p, b1p = fold_bn(g1, beta1, rm1, rv1)
    g2p, b2p = fold_bn(g2, beta2, rm2, rv2)
    whi = np.stack([W1[:128], W2[:128]]).astype(np.float16)
    wlo = np.stack([W1[128:], W2[128:]]).astype(np.float16)
    brow = np.stack([b1[None, :], b2[None, :]]).astype(np.float16)
    gam = np.stack([np.broadcast_to(g1p, (128, D)),
                    np.broadcast_to(g2p, (128, D))]).astype(np.float32)
    bet = np.stack([np.broadcast_to(b1p, (128, D)),
                    np.broadcast_to(b2p, (128, D))]).astype(np.float32)
    in_maps = []
    for k in range(NCORES):
        in_maps.append({
            "xbuf": xbuf, "idxt": pp["idx_arr"][k], "dcol": pp["d_arr"][k],
            "wcol": pp["w_arr"][k], "bcol": pp["bloc_arr"][k],
            "iot": iot, "idn": idn, "onesr": onesr,
            "whi": whi, "wlo": wlo, "brow": brow, "gam": gam, "bet": bet,
        })
    return nc, in_maps, pp


def combine(pp, outs):
    sums = np.zeros((pp["G"] + PB, pp["D"]), dtype=np.float32)
    for k in range(NCORES):
        o = outs[k]
        for b in range(pp["NB"]):
            bb = pp["base"][k * pp["NB"] + b]
            sums[bb:bb + PB] += o[b * PB:(b + 1) * PB]
    return (sums[:pp["G"]]
            / np.maximum(pp["cnts"], 1.0)[:, None]).astype(np.float32)



# revision 24
# speedup vs baseline: 47.8646x; 47.8646x over previous
"""Trainium2 Bass kernel for a 2-layer GCN (GCNConv+ReLU+BN x2, mean-pool).

Strategy (8 NeuronCores, SPMD):
- Dest-node sharding: each core owns NB=ceil(N/8/128) blocks of 128 nodes.
- Message passing out[c] = sum_e w_e * h[src_e] (w_e = dinv[r]*dinv[c],
  self-loops included as edges) is computed per dest block as a chain of
  one-hot matmuls on the TensorEngine: psum += E_t^T @ msg_t, where msg_t is
  128 source rows fetched with dma_gather (fp16 rows padded to 512B) and
  E_t[e, d] = (d_e == d) * w_e built on the VectorEngine.
- E matrices for a whole block (both halves) are built with TWO big
  tensor_tensor instructions using 0-stride broadcast APs (instead of one
  tensor_scalar per 128-edge tile).
- Gather calls are capped at 1024 indices (the SWDGE descriptor ring holds
  ~1024 descriptors per queue; larger calls deadlock awaiting ring space)
  and round-robin over 4 SWDGE queues. Padding indices are -1 so the Q7
  desc-gen kernel skips them; num_idxs_reg carries the per-core valid count
  (it must match the -1 trim or the ring bookkeeping desyncs and hangs).
- The weight multiply commutes with aggregation: per block, agg is
  transposed on the PE and multiplied by W' = W*diag(bn_scale) (fp16, f32
  accum; the BN scale folds into W since relu(z*g) == relu(z)*g for g>0),
  bias added via a ones-row matmul, then ReLU + psum evacuations on the
  otherwise-idle ScalarE, and the BN shift on the VectorE.
- Pool one-hot matrices are precomputed on the host (they depend only on
  `batch`) and streamed in as constants.
- Layer-1 results are written per-core and AllGathered (on-chip collective)
  into a shared buffer that layer 2 gathers from.
- Mean-pool: per block a one-hot P matmul reduces 128 nodes into <=128
  graph partials; the host sums overlapping block partials and divides by
  graph counts (the cross-core unshard step).
"""
import os
import numpy as np
from contextlib import ExitStack

import concourse.bacc as bacc
import concourse.bass as bass
import concourse.mybir as mybir
import concourse.tile as tile
from concourse.library_config import mlp
from concourse.bass_utils import run_bass_kernel_spmd

dt = mybir.dt
NCORES = 8
PB = 128          # nodes per dest block
EW = 256          # fp16 elements per padded row (512 bytes)
EPS = 1e-5
G_OUT = 2048      # number of graphs in the output


# ---------------------------------------------------------------- host prep
def preprocess(x, edge_index, batch, force_T=None):
    N, D = x.shape
    G = G_OUT
    NB = -(-N // (NCORES * PB))          # blocks per core
    S = NB * PB
    TOT = NCORES * S
    L = TOT // 2
    assert L <= 32768 and TOT - L <= 32768

    r = np.asarray(edge_index[0], dtype=np.int64)
    c = np.asarray(edge_index[1], dtype=np.int64)
    loops = np.arange(N, dtype=np.int64)
    r = np.concatenate([r, loops])
    c = np.concatenate([c, loops])
    deg = np.bincount(c, minlength=N).astype(np.float64)
    dinv = np.where(deg > 0, 1.0 / np.sqrt(deg), 0.0)
    w = (dinv[r] * dinv[c]).astype(np.float32)

    half = (r >= L).astype(np.int64)
    src = np.where(half == 0, r, r - L).astype(np.int64)
    blk = c // PB                         # global dest block 0..NCORES*NB-1
    d = (c % PB).astype(np.int64)

    core = blk // NB
    b_loc = blk % NB
    # group key: (core, half, local block); same-half blocks contiguous so a
    # single dma_gather can cover several consecutive blocks
    ngroups = NCORES * 2 * NB
    key = (core * 2 + half) * NB + b_loc
    # sort by (group, src): ascending source rows within each group give the
    # HBM gather far better page locality
    order = np.lexsort((src, key))
    src, d, w, key = src[order], d[order], w[order], key[order]
    counts = np.bincount(key, minlength=ngroups)
    T = max(1, int(-(-counts.max() // PB)))
    if force_T is not None:
        assert force_T >= T
        T = force_T
    gsz = T * PB

    src_p = np.zeros((ngroups, gsz), dtype=np.int16)
    d_p = np.zeros((ngroups, gsz), dtype=np.float32)
    w_p = np.zeros((ngroups, gsz), dtype=np.float32)
    starts = np.concatenate([[0], np.cumsum(counts)])
    for g in range(ngroups):
        n = counts[g]
        src_p[g, :n] = src[starts[g]:starts[g] + n]
        d_p[g, :n] = d[starts[g]:starts[g] + n]
        w_p[g, :n] = w[starts[g]:starts[g] + n]

    # gather-call splits: at most MAXI indices per dma_gather call
    MAXI = int(os.environ.get("GCN_MAXI", "1024"))
    tc_max = max(1, MAXI // PB)
    splits = []
    t0 = 0
    while t0 < T:
        tc = min(tc_max, T - t0)
        splits.append((t0, tc))
        t0 += tc
    NS = len(splits)

    # per-core device arrays; group-local index gl = h*NB + b
    G2 = 2 * NB
    negpad = os.environ.get("GCN_NEGPAD", "1") == "1"
    idx_arr = np.zeros((NCORES, 128, G2 * 8 * T), dtype=np.int16)
    # d/w packed block-major: [128, b, h, t] so one E-build covers both halves
    d_arr = np.zeros((NCORES, 128, NB * 2 * T), dtype=np.float32)
    w_arr = np.zeros((NCORES, 128, NB * 2 * T), dtype=np.float32)
    cnt_arr = np.zeros((NCORES, 1, G2 * NS), dtype=np.int32)
    for k in range(NCORES):
        for gl in range(G2):
            g = k * G2 + gl
            b_l = gl % NB
            h_l = gl // NB
            idxs = src_p[g].copy()
            n = counts[g]
            for s, (t0, tc) in enumerate(splits):
                if negpad and b_l >= 2 and s > 0:
                    # valid (non-padded) indices within this call; the Q7
                    # desc-gen kernel trims the trailing -1s and the decode
                    # side must be told the same count via num_idxs_reg.
                    v = min(max(n - t0 * PB, 0), tc * PB)
                else:
                    v = tc * PB
                cnt_arr[k, 0, gl * NS + s] = v
            if negpad and b_l >= 2:
                # trailing -1 padding: blocks 0-1 keep 0-padding so the first
                # uses of each gather pool buffer never read uninitialized
                # SBUF into the matmuls. Subcall 0 keeps 0-padding (its
                # num_idxs_reg is the full constant).
                s0_end = splits[0][1] * PB
                idxs[max(n, s0_end):] = -1
            # dma_gather index layout: index j lives at [j % 16, j // 16]
            wrapped = idxs.reshape(8 * T, 16).T                # [16, 8T]
            idx_arr[k, :, gl * 8 * T:(gl + 1) * 8 * T] = np.tile(wrapped, (8, 1))
            col = (b_l * 2 + h_l) * T
            d_arr[k, :, col:col + T] = d_p[g].reshape(T, PB).T
            w_arr[k, :, col:col + T] = w_p[g].reshape(T, PB).T

    # pooling: per (core, block) graph base + local graph ids
    batch_pad = np.full(TOT, -1, dtype=np.int64)
    batch_pad[:N] = np.asarray(batch, dtype=np.int64)
    blocks = batch_pad.reshape(NCORES * NB, PB)
    valid = blocks >= 0
    base = np.where(valid.any(axis=1),
                    np.where(valid, blocks, np.iinfo(np.int64).max).min(axis=1),
                    0)
    bloc = np.where(valid, blocks - base[:, None], -1).astype(np.float32)
    bloc_arr = bloc.reshape(NCORES, NB, PB).transpose(0, 2, 1).copy()  # [k,128,NB]
    # pool one-hot matrices P[k][l, b*128+g] = (bloc[l, b] == g), f16
    gids = np.arange(PB, dtype=np.float32)
    p_arr = (bloc_arr[:, :, :, None] == gids).astype(np.float16)
    p_arr = p_arr.reshape(NCORES, 128, NB * PB)

    cnts = np.bincount(np.asarray(batch, dtype=np.int64),
                       minlength=G).astype(np.float32)
    return dict(N=N, D=D, G=G, NB=NB, S=S, TOT=TOT, L=L, T=T, NS=NS,
                splits=splits, idx_arr=idx_arr, d_arr=d_arr, w_arr=w_arr,
                cnt_arr=cnt_arr, bloc_arr=bloc_arr, p_arr=p_arr, base=base,
                cnts=cnts)


def fold_bn(g, beta, rm, rv):
    gp = (g / np.sqrt(rv + EPS)).astype(np.float32)
    bp = (beta - rm * gp).astype(np.float32)
    return gp, bp


# ---------------------------------------------------------------- bass build
def build_nc(NB, T, D, TOT, S, L, NS, reps=1):
    f16, f32, i16 = dt.float16, dt.float32, dt.int16
    i32 = dt.int32
    G2 = 2 * NB
    NQ = int(os.environ.get("GCN_NQ", "4"))      # SWDGE queues (Q7 cpu pairs)
    MAXI = int(os.environ.get("GCN_MAXI", "1024"))  # max idxs per gather call
    SCR = int(os.environ.get("GCN_SCRATCH", "16384"))
    nc = bacc.Bacc("TRN2", target_bir_lowering=False, debug=False,
                   num_devices=NCORES, num_swdge_queues=NQ,
                   dynamic_dma_scratch_size=SCR)

    xbuf = nc.dram_tensor("xbuf", [TOT, EW], f16, kind="ExternalInput")
    idxt = nc.dram_tensor("idxt", [128, G2 * 8 * T], i16, kind="ExternalInput")
    dcol = nc.dram_tensor("dcol", [128, G2 * T], f32, kind="ExternalInput")
    wcol = nc.dram_tensor("wcol", [128, G2 * T], f32, kind="ExternalInput")
    pcol = nc.dram_tensor("pcol", [128, NB * 128], f16, kind="ExternalInput")
    cntd = nc.dram_tensor("cntd", [1, G2 * NS], i32, kind="ExternalInput")
    iot = nc.dram_tensor("iot", [128, 128], f16, kind="ExternalInput")
    idn = nc.dram_tensor("idn", [128, 128], f32, kind="ExternalInput")
    onesr = nc.dram_tensor("onesr", [1, 128], f16, kind="ExternalInput")
    whi = nc.dram_tensor("whi", [2, 128, D], f16, kind="ExternalInput")
    wlo = nc.dram_tensor("wlo", [2, D - 128, D], f16, kind="ExternalInput")
    brow = nc.dram_tensor("brow", [2, 1, D], f16, kind="ExternalInput")
    gam = nc.dram_tensor("gam", [2, 128, D], f32, kind="ExternalInput")
    bet = nc.dram_tensor("bet", [2, 128, D], f32, kind="ExternalInput")
    outp = nc.dram_tensor("outp", [NB * PB, D], f32, kind="ExternalOutput")
    h1sl = nc.dram_tensor("h1sl", [S, EW], f16, kind="Internal")
    h1f = nc.dram_tensor("h1f", [TOT, EW], f16, kind="Internal",
                         addr_space="Shared")

    DLO = D - 128
    with tile.TileContext(nc) as tc, ExitStack() as ctx:
        cp = ctx.enter_context(tc.tile_pool(name="consts", bufs=1))
        gp_ = ctx.enter_context(tc.tile_pool(name="gath", bufs=10))
        ep0 = ctx.enter_context(tc.tile_pool(name="eo", bufs=2))
        ep = ctx.enter_context(tc.tile_pool(name="onehot", bufs=2))
        pp_ = ctx.enter_context(tc.tile_pool(name="pmat", bufs=2))
        sp = ctx.enter_context(tc.tile_pool(name="stage", bufs=3))
        pagg = ctx.enter_context(tc.tile_pool(name="pagg", bufs=2, space="PSUM"))
        ptr = ctx.enter_context(tc.tile_pool(name="ptr", bufs=2, space="PSUM"))
        ph = ctx.enter_context(tc.tile_pool(name="ph", bufs=2, space="PSUM"))
        ppool = ctx.enter_context(tc.tile_pool(name="ppool", bufs=2, space="PSUM"))

        def load_const(name, dram, shape, dtype):
            t = cp.tile(shape, dtype, name=name)
            nc.sync.dma_start(t[:], dram)
            return t

        idxS = load_const("idxS", idxt[:, :], [128, G2 * 8 * T], i16)
        dS = load_const("dS", dcol[:, :], [128, G2 * T], f32)
        wS = load_const("wS", wcol[:, :], [128, G2 * T], f32)
        pS = load_const("pS", pcol[:, :], [128, NB * 128], f16)
        cntS = load_const("cntS", cntd[:, :], [1, G2 * NS], i32)
        iotS = load_const("iotS", iot[:, :], [128, 128], f16)
        idnS = load_const("idnS", idn[:, :], [128, 128], f32)
        onesS = load_const("onesS", onesr[:, :], [1, 128], f16)
        whiS = [load_const(f"whiS{l}", whi[l, :, :], [128, D], f16) for l in range(2)]
        wloS = [load_const(f"wloS{l}", wlo[l, :, :], [DLO, D], f16) for l in range(2)]
        browS = [load_const(f"browS{l}", brow[l, :, :], [1, D], f16) for l in range(2)]
        gamS = [load_const(f"gamS{l}", gam[l, :, :], [128, D], f32) for l in range(2)]
        betS = [load_const(f"betS{l}", bet[l, :, :], [128, D], f32) for l in range(2)]

        nc.gpsimd.load_library(mlp)
        cnt_regs = [nc.gpsimd.alloc_register(f"gcnt{i}") for i in range(4)]

        env = dict(NB=NB, T=T, D=D, TOT=TOT, S=S, L=L, NQ=NQ, MAXI=MAXI,
                   G2=G2, NS=NS, cntS=cntS, cnt_regs=cnt_regs,
                   xbuf=xbuf, h1f=h1f, h1sl=h1sl, outp=outp,
                   gp_=gp_, ep0=ep0, ep=ep, pp_=pp_, sp=sp,
                   pagg=pagg, ptr=ptr, ph=ph, ppool=ppool,
                   idxS=idxS, dS=dS, wS=wS, pS=pS, iotS=iotS, idnS=idnS,
                   onesS=onesS, whiS=whiS, wloS=wloS, browS=browS,
                   gamS=gamS, betS=betS)
        for rep in range(reps):
            env["rep"] = rep
            _do_body(nc, tc, env)

    nc.compile()
    return nc


def _do_body(nc, tc, env):
    """One full forward pass; env carries the tiles/pools from build_nc."""
    (NB, T, D, TOT, S, L, NQ, MAXI, rep) = (env[k] for k in
                                            ("NB", "T", "D", "TOT", "S", "L",
                                             "NQ", "MAXI", "rep"))
    (xbuf, h1f, h1sl, outp) = (env[k] for k in ("xbuf", "h1f", "h1sl", "outp"))
    (gp_, ep0, ep, pp_, sp, pagg, ptr, ph, ppool) = (env[k] for k in
        ("gp_", "ep0", "ep", "pp_", "sp", "pagg", "ptr", "ph", "ppool"))
    (idxS, dS, wS, pS, iotS, idnS, onesS) = (env[k] for k in
        ("idxS", "dS", "wS", "pS", "iotS", "idnS", "onesS"))
    (whiS, wloS, browS, gamS, betS) = (env[k] for k in
        ("whiS", "wloS", "browS", "gamS", "betS"))
    f16, f32 = dt.float16, dt.float32
    DLO = D - 128
    R = rep
    iotB2 = iotS[:, :].unsqueeze(1).broadcast_to((128, 2 * T, 128))
    cntS = env["cntS"]
    NS = env["NS"]
    # tile ranges per gather call: at most MAXI indices each
    tc_max = max(1, MAXI // PB)
    splits = []
    t0 = 0
    while t0 < T:
        tc = min(tc_max, T - t0)
        splits.append((t0, tc))
        t0 += tc
    assert len(splits) == NS
    qn = [0]
    if rep == 0:
        # warm-zero the gather pool slots: negpad-trimmed calls skip their
        # padded tail, leaving whatever the slot held — zeros after this,
        # never uninitialized SBUF (which can hold NaN patterns; the PE
        # propagates 0*NaN into the psum accumulation).
        tcm = max(1, MAXI // PB)
        for i in range(10):
            gw = gp_.tile([128, tcm, EW], dt.float16,
                          name=f"gtwarm_{R}_{i}", tag="gt")
            nc.vector.memset(gw[:], 0)
    for layer in range(2):
        src = xbuf if layer == 0 else h1f
        for b in range(NB):
            agg = pagg.tile([128, D], f32, name=f"agg_{R}_{layer}_{b}",
                            tag="agg")
            E = ep.tile([128, 2 * T, 128], f16,
                        name=f"E_{R}_{layer}_{b}", tag="E")
            E0 = ep0.tile([128, 2 * T, 128], f16,
                          name=f"E0_{R}_{layer}_{b}", tag="E0")
            col = b * 2 * T
            dB = dS[:, col:col + 2 * T].unsqueeze(2) \
                .broadcast_to((128, 2 * T, 128))
            wB = wS[:, col:col + 2 * T].unsqueeze(2) \
                .broadcast_to((128, 2 * T, 128))
            nc.vector.tensor_tensor(E0[:], iotB2, dB,
                                    op=mybir.AluOpType.is_equal)
            nc.vector.tensor_tensor(E[:], E0[:], wB,
                                    op=mybir.AluOpType.mult)
            for h in range(2):
                gl = h * NB + b
                in_ap = src[0:TOT, :] if h == 0 else src[L:TOT, :]
                subs = []
                for s, (t0, tc) in enumerate(splits):
                    gt = gp_.tile([128, tc, EW], f16,
                                  name=f"gt_{R}_{layer}_{gl}_{t0}", tag="gt")
                    ci = gl * NS + s
                    if s == 0:
                        vr = tc * PB
                    else:
                        vr = env["cnt_regs"][qn[0] % 4]
                        nc.gpsimd.reg_load(vr, cntS[0:1, ci:ci + 1])
                    nc.gpsimd.dma_gather(
                        gt[:], in_ap,
                        idxS[:, gl * 8 * T + t0 * 8:
                             gl * 8 * T + (t0 + tc) * 8],
                        tc * PB, vr, EW, single_packet=True,
                        queue_num=qn[0] % env["NQ"])
                    qn[0] += 1
                    subs.append((t0, tc, gt))
                for (t0, tc, gt) in subs:
                    for t in range(t0, t0 + tc):
                        nc.tensor.matmul(
                            agg[:], E[:, h * T + t, :], gt[:, t - t0, 0:D],
                            start=(h == 0 and t == 0),
                            stop=(h == 1 and t == T - 1))
            # epilogue: transpose agg, @W, +b, relu, BN affine
            aggS = sp.tile([128, D], f32, name=f"aggS_{R}_{layer}_{b}",
                           tag="aggS")
            nc.scalar.activation(aggS[:], agg[:],
                                 mybir.ActivationFunctionType.Copy)
            psT = ptr.tile([128, 256], f32, name=f"psT_{R}_{layer}_{b}",
                           tag="psT")
            nc.tensor.transpose(psT[:, 0:128], aggS[:, 0:128], idnS[:])
            nc.tensor.transpose(psT[0:DLO, 128:256], aggS[:, 128:D], idnS[:])
            t1 = sp.tile([128, 128], f16, name=f"t1_{R}_{layer}_{b}", tag="t1")
            nc.scalar.activation(t1[:], psT[:, 0:128],
                                 mybir.ActivationFunctionType.Copy)
            t2 = sp.tile([DLO, 128], f16, name=f"t2_{R}_{layer}_{b}", tag="t2")
            nc.scalar.activation(t2[:], psT[0:DLO, 128:256],
                                 mybir.ActivationFunctionType.Copy)
            zps = ph.tile([128, D], f32, name=f"zps_{R}_{layer}_{b}", tag="zps")
            nc.tensor.matmul(zps[:], t1[:], whiS[layer][:],
                             start=True, stop=False)
            nc.tensor.matmul(zps[:], t2[:], wloS[layer][:],
                             start=False, stop=False)
            nc.tensor.matmul(zps[:], onesS[:], browS[layer][:],
                             start=False, stop=True)
            rl = sp.tile([128, D], f32, name=f"rl_{R}_{layer}_{b}", tag="rl")
            nc.scalar.activation(rl[:], zps[:],
                                 mybir.ActivationFunctionType.Relu)
            hS = sp.tile([128, D], f16, name=f"hS_{R}_{layer}_{b}", tag="hS")
            nc.vector.tensor_add(hS[:], rl[:], betS[layer][:])
            if layer == 0:
                nc.sync.dma_start(h1sl[b * PB:(b + 1) * PB, 0:D], hS[:])
            else:
                pps = ppool.tile([128, D], f32, name=f"pps_{R}_{b}", tag="pps")
                nc.tensor.matmul(pps[:], pS[:, b * 128:(b + 1) * 128], hS[:],
                                 start=True, stop=True)
                po = sp.tile([128, D], f32, name=f"po_{R}_{b}", tag="po")
                nc.scalar.activation(po[:], pps[:],
                                     mybir.ActivationFunctionType.Copy)
                nc.sync.dma_start(outp[b * PB:(b + 1) * PB, :], po[:])
        if layer == 0:
            nc.gpsimd.collective_compute(
                "AllGather", mybir.AluOpType.bypass,
                replica_groups=[list(range(NCORES))],
                ins=[h1sl[:, :].opt()], outs=[h1f[:, :].opt()])


# ---------------------------------------------------------------- entry
_NC_CACHE = {}


def kernel(x, edge_index, batch, W1, b1, W2, b2,
           g1, beta1, rm1, rv1, g2, beta2, rm2, rv2):
    nc, in_maps, pp = prepare(x, edge_index, batch, W1, b1, W2, b2,
                              g1, beta1, rm1, rv1, g2, beta2, rm2, rv2)
    res = run_bass_kernel_spmd(nc, in_maps, core_ids=list(range(NCORES)))
    return combine(pp, [res.results[k]["outp"] for k in range(NCORES)])


def prepare(x, edge_index, batch, W1, b1, W2, b2,
            g1, beta1, rm1, rv1, g2, beta2, rm2, rv2):
    """Build (nc, in_maps, pp) without running — used by the benchmark."""
    x = np.asarray(x, dtype=np.float32)
    pp = preprocess(x, np.asarray(edge_index), np.asarray(batch))
    D = pp["D"]
    key = (pp["NB"], pp["T"], D, pp["TOT"], pp["S"], pp["L"], pp["NS"])
    if key not in _NC_CACHE:
        _NC_CACHE[key] = build_nc(*key)
    nc = _NC_CACHE[key]

    xbuf = np.zeros((pp["TOT"], EW), dtype=np.float16)
    xbuf[:pp["N"], :D] = x.astype(np.float16)
    iot = np.broadcast_to(np.arange(128, dtype=np.float16), (128, 128)).copy()
    idn = np.eye(128, dtype=np.float32)
    onesr = np.ones((1, 128), dtype=np.float16)
    g1p, b1p = fold_bn(g1, beta1, rm1, rv1)
    g2p, b2p = fold_bn(g2, beta2, rm2, rv2)
    assert (g1p > 0).all() and (g2p > 0).all(), \
        "gamma fold into W needs positive BN scale (relu(z*g) == relu(z)*g)"
    whi = np.stack([W1[:128] * g1p, W2[:128] * g2p]).astype(np.float16)
    wlo = np.stack([W1[128:] * g1p, W2[128:] * g2p]).astype(np.float16)
    brow = np.stack([(b1 * g1p)[None, :],
                     (b2 * g2p)[None, :]]).astype(np.float16)
    gam = np.stack([np.broadcast_to(g1p, (128, D)),
                    np.broadcast_to(g2p, (128, D))]).astype(np.float32)
    bet = np.stack([np.broadcast_to(b1p, (128, D)),
                    np.broadcast_to(b2p, (128, D))]).astype(np.float32)
    in_maps = []
    for k in range(NCORES):
        in_maps.append({
            "xbuf": xbuf, "idxt": pp["idx_arr"][k], "dcol": pp["d_arr"][k],
            "wcol": pp["w_arr"][k], "pcol": pp["p_arr"][k],
            "cntd": pp["cnt_arr"][k],
            "iot": iot, "idn": idn, "onesr": onesr,
            "whi": whi, "wlo": wlo, "brow": brow, "gam": gam, "bet": bet,
        })
    return nc, in_maps, pp


def combine(pp, outs):
    sums = np.zeros((pp["G"] + PB, pp["D"]), dtype=np.float32)
    for k in range(NCORES):
        o = outs[k]
        for b in range(pp["NB"]):
            bb = pp["base"][k * pp["NB"] + b]
            sums[bb:bb + PB] += o[b * PB:(b + 1) * PB]
    return (sums[:pp["G"]]
            / np.maximum(pp["cnts"], 1.0)[:, None]).astype(np.float32)
